# revision 41
# baseline (speedup 1.0000x reference)
"""CoAttention kernel for 8 TRN2 NeuronCores.

Sharding: batch (4) x role (2) = 8 cores, no collectives.
  core 2b   ("query" role):    computes out2[b] (query_att path)
  core 2b+1 ("exemplar" role): computes out1[b] (exemplar_att path)

Both roles run the SAME program on different data, exploiting the symmetry
  out2 = W2a @ gate(softmax_n(A) applied to ex)      + W2b @ input_2
  out1 = W1a @ gate(softmax_m(A^T) applied to q)     + W1b @ input_1
with A(X, Y, Wh) = (Wh @ X)^T @ Y.  Role Q: X=input_1, Y=input_2, Wh=W_e.
Role E: X=input_2, Y=input_1, Wh=W_e^T (then A' = A^T and the "column"
softmax of A' is the row softmax of A).

v2 (vs the fp32r baseline): all-bf16 matmul datapath. On HW a self-loading
fp32 matmul pays a serial 128-col weight load at 1.2 GHz (~107 ns) on top of
the 512-col stream (~213 ns); bf16 gets FWL (2 cols/cycle, ~53 ns) and
halves DMA traffic. Measured via micro2.py: 323 ns/MM fp32r vs 266 ns/MM
bf16 in a pure stream. The exp is batched over [128, 1024] PSUM pairs
(fewer ACT bubbles), colsum partials accumulate in bf16 (DVE 2-byte fast
mode), and the per-chunk gate/output tail is software-pipelined into the
next chunk's A/U matmul stream so the PE never idles on the serial
colsum->recip->gate->scale chain.

Per-core program (C=256, n = X pixels, m = Y pixels):
  EC = Wh @ X                    [C, n]  (bf16)
  for each m-chunk (512):
    for each nj pair (2x128 rows):
      A_pair = EC_chunk^T @ Y_chunk   x4  (PE, bf16, 2-bank PSUM group)
      P_pair = exp(A_pair - KEXP)         (ACT, one [128,1024] instr, bf16)
      cs    += P_pair                     (DVE bf16 fast mode, 2 adds)
      U0/U1 += X_chunk @ P_sub        x4  (PE, accumulated in PSUM)
    tail (pipelined into next chunk's stream):
      colsum = ones^T @ cs; recip         (PE, DVE)
      gdot   = gate_w^T @ U               (PE)
      scale  = sigmoid(gdot*recip)*recip  (ACT/DVE, [1,512])
      bcast  = ones_col @ scale           (PE outer product)
      gated  = U * bcast                  (DVE)
      out    = WaT^T @ gated + WbT^T @ Y  (PE) -> DMA
"""

import numpy as np
import ml_dtypes

import concourse.bass as bass
import concourse.bacc as bacc
import concourse.tile as tile
from concourse import mybir
from concourse import bass_utils

F32 = mybir.dt.float32
BF16 = mybir.dt.bfloat16

B = 4
C = 256
H = 64
W = 64
HW = H * W
KEXP = 20.0  # constant subtracted before exp (softmax-invariant)

TRACE = False

_COMPILED = {}


def _build_nc(n_pix, m_pix, rep=1):
    nc = bacc.Bacc(
        "TRN2",
        target_bir_lowering=False,
        debug=False,
        enable_asserts=True,
        num_devices=8,
    )
    X = nc.dram_tensor("x", [C, n_pix], BF16, kind="ExternalInput").ap()
    XT = nc.dram_tensor("xt", [n_pix, C], BF16, kind="ExternalInput").ap()
    Y = nc.dram_tensor("y", [C, m_pix], BF16, kind="ExternalInput").ap()
    WHT = nc.dram_tensor("wht", [C, C], BF16, kind="ExternalInput").ap()
    WAT = nc.dram_tensor("wat", [C, C], BF16, kind="ExternalInput").ap()
    WBT = nc.dram_tensor("wbt", [C, C], BF16, kind="ExternalInput").ap()
    GW = nc.dram_tensor("gw", [C, 1], BF16, kind="ExternalInput").ap()
    OUT = nc.dram_tensor("out", [C, m_pix], F32, kind="ExternalOutput").ap()

    NCH = n_pix // 128  # 32 n-chunks of 128
    NG = NCH // 2       # 16 groups of 2 n-chunks
    MCH = m_pix // 512  # 8 m-chunks
    NK = n_pix // 512
    Exp = mybir.ActivationFunctionType.Exp
    Copy = mybir.ActivationFunctionType.Copy

    with tile.TileContext(nc) as tc:
        with (
            nc.allow_low_precision(reason="bf16 matmul datapath"),
            tc.tile_pool(name="persist", bufs=1) as persist,
            # [128, 2, 512] f32 = 2 PSUM banks per A group, double buffered
            tc.tile_pool(name="psA", bufs=2, space=bass.MemorySpace.PSUM) as psA,
            tc.tile_pool(name="psU", bufs=1, space=bass.MemorySpace.PSUM) as psU,
            tc.tile_pool(name="psO", bufs=1, space=bass.MemorySpace.PSUM) as psO,
            tc.tile_pool(name="psBC", bufs=1, space=bass.MemorySpace.PSUM) as psBC,
            tc.tile_pool(name="pwork", bufs=3) as pwork,
            tc.tile_pool(name="accp", bufs=2) as accp,
            tc.tile_pool(name="upool", bufs=2) as upool,
            tc.tile_pool(name="opool", bufs=2) as opool,
            tc.tile_pool(name="small", bufs=2) as small,
        ):
            # ---- persistent loads, ordered+chunked by first consumption ----
            Xr = X.rearrange("(ci p) n -> p ci n", p=128)
            Yr = Y.rearrange("(ci p) m -> p ci m", p=128)
            XTr = XT.rearrange("(a p) c -> p a c", p=128)
            wht_sb = persist.tile([128, 2, C], BF16)
            nc.sync.dma_start(out=wht_sb, in_=WHT.rearrange("(ci p) d -> p ci d", p=128))
            ones_col = persist.tile([128, 1], BF16)
            nc.vector.memset(ones_col, 1.0)
            ones_row = persist.tile([1, 128], BF16)
            nc.vector.memset(ones_row, 1.0)
            x_sb = persist.tile([128, 2, n_pix], BF16)
            for nk2 in range(NK // 2):
                nsl = slice(nk2 * 1024, (nk2 + 1) * 1024)
                for ci in range(2):
                    nc.sync.dma_start(out=x_sb[:, ci, nsl], in_=Xr[:, ci, nsl])
            y_sb = persist.tile([128, 2, m_pix], BF16)
            for ci in range(2):
                nc.sync.dma_start(out=y_sb[:, ci, 0:512], in_=Yr[:, ci, 0:512])
            xT_sb = persist.tile([128, NCH, C], BF16)
            for a in range(0, NCH, 4):
                nc.sync.dma_start(out=xT_sb[:, a:a + 4, :], in_=XTr[:, a:a + 4, :])
            for mk2 in range(MCH // 2):
                msl_ = slice(max(mk2 * 1024, 512), (mk2 + 1) * 1024)
                for ci in range(2):
                    nc.sync.dma_start(out=y_sb[:, ci, msl_], in_=Yr[:, ci, msl_])
            wat_sb = persist.tile([128, 2, C], BF16)
            nc.sync.dma_start(out=wat_sb, in_=WAT.rearrange("(ci p) o -> p ci o", p=128))
            wbt_sb = persist.tile([128, 2, C], BF16)
            nc.sync.dma_start(out=wbt_sb, in_=WBT.rearrange("(ci p) o -> p ci o", p=128))
            gw_sb = persist.tile([128, 2, 1], BF16)
            nc.sync.dma_start(out=gw_sb, in_=GW.rearrange("(ci p) o -> p ci o", p=128))
            negk128 = persist.tile([128, 1], F32)
            nc.vector.memset(negk128, -KEXP)
            zero1 = persist.tile([1, 1], F32)
            nc.vector.memset(zero1, 0.0)
            ec_sb = persist.tile([128, 2, n_pix], BF16)

            # ---- EC = Wh @ X (batched pairs through 2-bank PSUM groups) ----
            # nk2-major so ec columns complete in order across BOTH dj halves,
            # letting chunk 0's A-matmuls start before all of EC is done.
            for nk2 in range(NK // 2):
                for dj in range(2):
                    ec_ps = psA.tile([128, 2, 512], F32, tag="a")
                    for k in range(2):
                        nsl = slice((nk2 * 2 + k) * 512, (nk2 * 2 + k + 1) * 512)
                        for ci in range(2):
                            nc.tensor.matmul(
                                ec_ps[:, k, :],
                                wht_sb[:, ci, dj * 128:(dj + 1) * 128],
                                x_sb[:, ci, nsl],
                                start=(ci == 0),
                                stop=(ci == 1),
                            )
                    for k in range(2):
                        nsl = slice((nk2 * 2 + k) * 512, (nk2 * 2 + k + 1) * 512)
                        nc.vector.tensor_copy(ec_sb[:, dj, nsl], ec_ps[:, k, :])

            # ---- main loop over m-chunks (rep>1 = timing-only replay via a
            # hardware For_i around the whole pass) ----
            # Tail work of chunk j is emitted interleaved into chunk j+1's
            # group stream (slots below) so the PE never waits on the serial
            # colsum->recip->gate->scale->out chain.
            mj_seq = list(range(MCH))

            def emit_tail(st, slot):
                """Emit one stage of the deferred tail for chunk state `st`."""
                mj = st["mj"]
                msl = st["msl"]
                if slot == 0:
                    # copy U out of PSUM (ACT + DVE in parallel)
                    u_sb0 = upool.tile([128, 512], BF16, tag="usb0")
                    u_sb1 = upool.tile([128, 512], BF16, tag="usb1")
                    nc.scalar.activation(u_sb0, st["u_ps0"], Copy)
                    nc.vector.tensor_copy(u_sb1, st["u_ps1"])
                    st["u_sb"] = (u_sb0, u_sb1)
                elif slot == 1:
                    cs_ps = psO.tile([1, 512], F32, tag="o")
                    nc.tensor.matmul(cs_ps, ones_col, st["cs_acc"])
                    recip_sb = small.tile([1, 512], F32, tag="recip")
                    nc.vector.reciprocal(recip_sb, cs_ps)
                    st["recip"] = recip_sb
                elif slot == 2:
                    gd_ps = psO.tile([1, 512], F32, tag="o")
                    u_sb0, u_sb1 = st["u_sb"]
                    nc.tensor.matmul(gd_ps, gw_sb[:, 0, :], u_sb0, start=True, stop=False)
                    nc.tensor.matmul(gd_ps, gw_sb[:, 1, :], u_sb1, start=False, stop=True)
                    # scale vector: sigmoid(gdot/colsum)/colsum
                    t_sb = small.tile([1, 512], F32, tag="t")
                    nc.vector.tensor_mul(t_sb, gd_ps, st["recip"])
                    e_sb = small.tile([1, 512], F32, tag="e")
                    nc.scalar.activation(e_sb, t_sb, Exp, bias=zero1, scale=-1.0)
                    ep1_sb = small.tile([1, 512], F32, tag="ep1")
                    nc.vector.tensor_scalar_add(ep1_sb, e_sb, 1.0)
                    g_sb = small.tile([1, 512], F32, tag="g")
                    nc.vector.reciprocal(g_sb, ep1_sb)
                    scale_sb = small.tile([1, 512], BF16, tag="scale")
                    nc.vector.tensor_mul(scale_sb, g_sb, st["recip"])
                    st["scale"] = scale_sb
                elif slot == 3:
                    # broadcast scale along partitions via outer product
                    bc_ps = psBC.tile([128, 512], F32, tag="bc")
                    nc.tensor.matmul(bc_ps, ones_row, st["scale"])
                    gated0 = upool.tile([128, 512], BF16, tag="gated0")
                    gated1 = upool.tile([128, 512], BF16, tag="gated1")
                    u_sb0, u_sb1 = st["u_sb"]
                    nc.vector.tensor_mul(gated0, u_sb0, bc_ps)
                    nc.vector.tensor_mul(gated1, u_sb1, bc_ps)
                    st["gated"] = (gated0, gated1)
                else:
                    # final 1x1 conv half: out = WaT^T @ gated + WbT^T @ Y
                    oj = slot - 4
                    osl = slice(oj * 128, (oj + 1) * 128)
                    o_ps = psO.tile([128, 512], F32, tag="o", name="o_ps")
                    for ci in range(2):
                        nc.tensor.matmul(
                            o_ps, wat_sb[:, ci, osl], st["gated"][ci],
                            start=(ci == 0), stop=False)
                    for ci in range(2):
                        nc.tensor.matmul(
                            o_ps, wbt_sb[:, ci, osl], y_sb[:, ci, msl],
                            start=False, stop=(ci == 1))
                    o_sb = opool.tile([128, 512], F32, tag="osb")
                    nc.scalar.activation(o_sb, o_ps, Copy)
                    if st["store"]:
                        nc.sync.dma_start(out=OUT[osl, msl], in_=o_sb)

            TAIL_SLOTS = 6  # slots 0..5; 1..5 emitted at group boundaries

            def emit_pass():
              prev = None
              for it, mj in enumerate(mj_seq):
                msl = slice(mj * 512, (mj + 1) * 512)
                st = {"mj": mj, "msl": msl, "store": True}
                st["u_ps0"] = psU.tile([128, 512], F32, tag="u0", name="u_ps0")
                st["u_ps1"] = psU.tile([128, 512], F32, tag="u1", name="u_ps1")
                st["cs_acc"] = accp.tile([128, 512], BF16, name="cs_acc")
                if prev is not None:
                    emit_tail(prev, 0)  # u copies must precede first U matmul
                # A-matmul groups with the U-matmuls pipelined one group
                # behind, so the PE never waits on the exp of the group it
                # just produced. (Two groups of lag measured WORSE on HW:
                # 332 vs 330 us/pass.)
                p_tiles = {}
                for g in range(NG + 1):
                    if g < NG:
                        a_ps = psA.tile([128, 2, 512], F32, tag="a")
                        for k in range(2):
                            nj = g * 2 + k
                            nsl128 = slice(nj * 128, (nj + 1) * 128)
                            for di in range(2):
                                nc.tensor.matmul(
                                    a_ps[:, k, :],
                                    ec_sb[:, di, nsl128],
                                    y_sb[:, di, msl],
                                    start=(di == 0),
                                    stop=(di == 1),
                                )
                        p_sb = pwork.tile([128, 2, 512], BF16, tag="p")
                        nc.scalar.activation(p_sb, a_ps, Exp, bias=negk128, scale=1.0)
                        if g == 0:
                            nc.vector.tensor_copy(st["cs_acc"], p_sb[:, 0, :])
                        else:
                            nc.vector.tensor_add(st["cs_acc"], st["cs_acc"], p_sb[:, 0, :])
                        nc.vector.tensor_add(st["cs_acc"], st["cs_acc"], p_sb[:, 1, :])
                        p_tiles[g] = p_sb
                    if g >= 1:
                        pg = p_tiles.pop(g - 1)
                        for k in range(2):
                            nj = (g - 1) * 2 + k
                            first = nj == 0
                            last = nj == NCH - 1
                            nc.tensor.matmul(
                                st["u_ps0"], xT_sb[:, nj, 0:128], pg[:, k, :],
                                start=first, stop=last)
                            nc.tensor.matmul(
                                st["u_ps1"], xT_sb[:, nj, 128:256], pg[:, k, :],
                                start=first, stop=last)
                    if prev is not None and 2 <= g <= TAIL_SLOTS:
                        emit_tail(prev, g - 1)
                prev = st
              # epilogue: drain the last chunk's tail
              for slot in range(TAIL_SLOTS):
                emit_tail(prev, slot)

            if rep == 1:
                emit_pass()
            else:
                # timing replay: hardware loop around the whole pass
                with tc.For_i(0, rep, 1, hint_engines=(
                        mybir.EngineType.PE, mybir.EngineType.Activation,
                        mybir.EngineType.DVE, mybir.EngineType.SP)):
                    emit_pass()

    nc.compile()
    return nc


def _get_compiled(n_pix, m_pix, rep=1):
    key = (n_pix, m_pix, rep)
    if key not in _COMPILED:
        _COMPILED[key] = _build_nc(n_pix, m_pix, rep)
    return _COMPILED[key]


def _in_maps(input_1, input_2, W_e, gate_w, W1, W2):
    bf = ml_dtypes.bfloat16
    ex = np.asarray(input_1, dtype=np.float32).reshape(B, C, HW)
    q = np.asarray(input_2, dtype=np.float32).reshape(B, C, HW)
    W_e = np.asarray(W_e, dtype=np.float32)
    gate_w = np.asarray(gate_w, dtype=np.float32).reshape(C, 1)
    W1 = np.asarray(W1, dtype=np.float32)
    W2 = np.asarray(W2, dtype=np.float32)

    def c(a):
        return np.ascontiguousarray(a).astype(bf)

    exb = [c(ex[b]) for b in range(B)]
    qb = [c(q[b]) for b in range(B)]
    exTb = [c(ex[b].T) for b in range(B)]
    qTb = [c(q[b].T) for b in range(B)]
    shared = {"gw": c(gate_w)}
    maps = []
    for b in range(B):
        # role Q -> out2[b]
        maps.append({
            "x": exb[b], "xt": exTb[b], "y": qb[b],
            "wht": c(W_e.T), "wat": c(W2[:, :C].T), "wbt": c(W2[:, C:].T),
            **shared,
        })
        # role E -> out1[b]
        maps.append({
            "x": qb[b], "xt": qTb[b], "y": exb[b],
            "wht": c(W_e), "wat": c(W1[:, :C].T), "wbt": c(W1[:, C:].T),
            **shared,
        })
    return maps


def kernel(input_1, input_2, W_e, gate_w, W1, W2):
    nc = _get_compiled(HW, HW)
    maps = _in_maps(input_1, input_2, W_e, gate_w, W1, W2)
    res = bass_utils.run_bass_kernel_spmd(
        nc, maps, core_ids=list(range(8)), trace=TRACE
    )
    kernel.last_results = res
    out1 = np.stack([res.results[2 * b + 1]["out"] for b in range(B)])
    out2 = np.stack([res.results[2 * b]["out"] for b in range(B)])
    return out1.reshape(B, C, H, W), out2.reshape(B, C, H, W)
